# revision 2
# baseline (speedup 1.0000x reference)
"""Trainium2 Bass kernel for nn_DiscreteAutoregressiveFlow (sampling, forward).

Math: `inputs` is an exact one-hot [B, L, V] tensor. For a row holding token v:
  net = W[v] + b                      (exact: one-hot @ W picks a row)
  loc = one_hot(argmax(net[:V]));  scale = one_hot(argmax(net[V:]))
  one_hot_multiply -> one-hot at (scale_tok*v) % V   (zero row if scale_tok==0)
  one_hot_add      -> one-hot at (scale_tok*v + loc_tok) % V
So out[row] = one_hot(cmap[v]) with a host-precomputed 64-entry map
(sentinel >= V encodes the zero row). The straight-through softmax residuals
and FFT noise in the reference are O(1e-7) and vanish in norm relative error.

Device pipeline per 128x(R*64) chunk (pure streaming, memory-bound):
  xb   = SWDGE cast-DMA in (gpsimd ring): f32 HBM -> bf16 SBUF, free convert
  prod = xb + cmapf                   (DVE TT add, all-bf16 -> 2x mode)
  m    = reduce_max(prod, inner V)    (DVE, bf16 accum -> 2x) = 1 + cmap[tok]/128
  out  = is_equal(iotaf, m)           (DVE, broadcast -> 1x) -> f32 one-hot rows
  HWDGE DMA-out on the scalar (ACT) ring.
All values involved are exact in bf16 (c <= 127 with 2^-7 scaling), so the
comparison is exact. Three DMA streams ride three different issue rings
(gpsimd SWDGE in / scalar HWDGE out / sync HWDGE constants) so the FIFO
ordering of one ring never stalls another; the baseline had all DMAs on the
sync ring, serializing chunk c's input behind chunk c-1's compute.
Sharding: pure data parallel over B*L rows, 8 cores, no collectives.
"""

import numpy as np
import ml_dtypes

V = 64
P = 128
N_CORES = 8
B, L = 16, 8192
ROWS = B * L                      # 131072
ROWS_PER_CORE = ROWS // N_CORES   # 16384
SENTINEL = 100.0
EPS = 1.0 / 128.0

# rows per partition per chunk; chunk = [128, R*64] f32 = R*32KB in DRAM
R = 16

_CACHE = {}


def _build_nc(rows_per_core: int, r: int):
    import concourse.bacc as bacc
    import concourse.mybir as mybir
    from concourse.bass import broadcast_tensor_aps
    from concourse.tile import TileContext

    f32 = mybir.dt.float32
    bf16 = mybir.dt.bfloat16
    fd = r * V
    chunk_rows = P * r
    n_chunks = rows_per_core // chunk_rows
    assert rows_per_core % chunk_rows == 0

    # Bacc (not raw Bass): its compile() runs generate_event_semaphores(),
    # which legalizes multi-wait instructions for TRN2 (1 wait per instr).
    nc = bacc.Bacc("TRN2", target_bir_lowering=False, name="daf_onehot")
    x = nc.dram_tensor("x", [rows_per_core, V], f32, kind="ExternalInput")
    cmapf = nc.dram_tensor("cmapf", [P, fd], bf16, kind="ExternalInput")
    iotaf = nc.dram_tensor("iotaf", [P, fd], bf16, kind="ExternalInput")
    y = nc.dram_tensor("y", [rows_per_core, V], f32, kind="ExternalOutput")

    xv = x.rearrange("(c p r) v -> c p (r v)", p=P, r=r)
    yv = y.rearrange("(c p r) v -> c p (r v)", p=P, r=r)

    with TileContext(nc) as tc:
        with (
            tc.tile_pool(name="const", bufs=1) as constp,
            tc.tile_pool(name="io", bufs=n_chunks) as iop,
            tc.tile_pool(name="work", bufs=n_chunks) as workp,
        ):
            cmap_t = constp.tile([P, fd], bf16, tag="cmapf")
            iota_t = constp.tile([P, fd], bf16, tag="iotaf")
            nc.sync.dma_start(cmap_t[:], cmapf[:])
            nc.sync.dma_start(iota_t[:], iotaf[:])
            if3 = iota_t[:].rearrange("p (r v) -> p r v", v=V)

            for ci in range(n_chunks):
                # SWDGE cast-DMA: reads f32 from HBM, writes bf16 to SBUF.
                # The convert rides the SDMA datapath - no engine time.
                xb = iop.tile([P, fd], bf16, tag="x")
                nc.gpsimd.dma_start(xb[:], xv[ci])

                prod = workp.tile([P, fd], bf16, tag="prod")
                nc.vector.tensor_tensor(
                    prod[:], xb[:], cmap_t[:], op=mybir.AluOpType.add
                )
                p3 = prod[:].rearrange("p (r v) -> p r v", v=V)

                # bf16 accumulator keeps every operand 2-byte/stride-1 so the
                # reduce runs in the DVE 2x mode; all values are exact in bf16.
                c_t = workp.tile([P, r], bf16, tag="c")
                nc.vector.tensor_reduce(
                    c_t[:], p3, axis=mybir.AxisListType.X, op=mybir.AluOpType.max
                )

                out_t = iop.tile([P, fd], f32, tag="out")
                o3 = out_t[:].rearrange("p (r v) -> p r v", v=V)
                c3 = c_t[:].rearrange("p (r one) -> p r one", one=1)
                c3_b, _ = broadcast_tensor_aps(c3, o3)
                nc.vector.tensor_tensor(o3, if3, c3_b, op=mybir.AluOpType.is_equal)

                nc.scalar.dma_start(yv[ci], out_t[:])

    # Bacc.finalize runs compile(): wait-splitting (generate_event_semaphores),
    # register allocation, nop fusion. run_bass_via_pjrt serializes nc.m as-is,
    # so this must happen here.
    nc.finalize()
    return nc


def _get_nc(rows_per_core=ROWS_PER_CORE, r=R):
    key = (rows_per_core, r)
    if key not in _CACHE:
        _CACHE[key] = _build_nc(rows_per_core, r)
    return _CACHE[key]


def _host_cmap(W: np.ndarray, b: np.ndarray) -> np.ndarray:
    """64-entry map token -> output one-hot index (or sentinel for zero row)."""
    net = W.astype(np.float32) + b.astype(np.float32)[None, :]   # [V, 2V]
    loc_tok = np.argmax(net[:, :V], axis=1)                      # [V]
    scale_tok = np.argmax(net[:, V:], axis=1)                    # [V]
    t = (scale_tok * np.arange(V, dtype=np.int64) + loc_tok) % V
    return np.where(scale_tok == 0, SENTINEL, t.astype(np.float64)).astype(
        np.float32
    )


def _host_tables(W: np.ndarray, b: np.ndarray):
    """bf16 [P, R*V] tiles of cmap*eps and 1 + iota*eps (all exact in bf16)."""
    cmap_eps = _host_cmap(W, b) * np.float32(EPS)
    iota_eps = 1.0 + np.arange(V, dtype=np.float32) * np.float32(EPS)
    cmapf = np.tile(cmap_eps[None, :], (P, R)).astype(ml_dtypes.bfloat16)
    iotaf = np.tile(iota_eps[None, :], (P, R)).astype(ml_dtypes.bfloat16)
    return cmapf, iotaf


def _in_maps(inputs: np.ndarray, W: np.ndarray, b: np.ndarray):
    x = np.ascontiguousarray(inputs.astype(np.float32, copy=False).reshape(ROWS, V))
    cmapf, iotaf = _host_tables(W, b)
    return [
        {
            "x": x[c * ROWS_PER_CORE : (c + 1) * ROWS_PER_CORE],
            "cmapf": cmapf,
            "iotaf": iotaf,
        }
        for c in range(N_CORES)
    ]


def kernel(inputs: np.ndarray, W: np.ndarray, b: np.ndarray) -> np.ndarray:
    from concourse import bass_utils

    nc = _get_nc()
    in_maps = _in_maps(inputs, W, b)
    res = bass_utils.run_bass_kernel_spmd(nc, in_maps, core_ids=list(range(N_CORES)))
    y = np.concatenate([r["y"] for r in res.results], axis=0)
    return y.reshape(inputs.shape).astype(inputs.dtype, copy=False)
